# revision 1
# baseline (speedup 1.0000x reference)
"""Trainium2 Bass kernel: 3x3 'same' cross-correlation on a [1,1,8192,8192] fp32 image.

Strategy (8 NeuronCores, row-sharded, memory-bound target):
  - Host: pad image rows by 1 top/bottom, shard into 8 overlapping [1026, 8192]
    row-shards (1 halo row each side). Kernel values arrive at trace time, so
    the Bass program is specialized to the nonzero taps of the 3x3 kernel.
  - Device (per core): for each tile of R output rows, load a single [R+nb, W]
    image tile A (nb = kernel row-span - 1). Vertical taps are applied with one
    banded matmul per nonzero kernel *column* (PE shifts across partitions;
    free-dim shifts handle the kernel column offset on the rhs AP). One tap is
    pulled out of the matmuls and fused into the PSUM->SBUF drain as a DVE
    scalar_tensor_tensor (out = A_shifted * w + psum). Output DMAs straight
    from SBUF.
  - Everything reads/writes HBM exactly once (+0.9% tile-overlap halo), which
    is the roofline for this memory-bound problem.
"""

import numpy as np

import concourse.bass as bass
import concourse.mybir as mybir
from concourse import bacc
from concourse import bass_utils
from concourse import tile

H = 8192
W = 8192
N_CORES = 8
RPC = H // N_CORES  # rows per core

F32 = mybir.dt.float32
F32R = mybir.dt.float32r


def _nonzero_taps(kern3: np.ndarray):
    """[(j, i, w)] for nonzero entries of the 3x3 kernel."""
    return [
        (j, i, float(kern3[j, i]))
        for j in range(kern3.shape[0])
        for i in range(kern3.shape[1])
        if kern3[j, i] != 0.0
    ]


def _band_matrix(col_taps, jmin, k_rows, out_rows):
    """lhsT [k_rows, out_rows] with B[k, p] = w for each (j, w) in col_taps
    where k = p + (j - jmin). matmul computes psum[p,:] = sum_k B[k,p]*A[k,:]."""
    B = np.zeros((k_rows, out_rows), dtype=np.float32)
    for j, w in col_taps:
        d = j - jmin
        for p in range(out_rows):
            k = p + d
            if 0 <= k < k_rows:
                B[k, p] = w
    return B


def build_program(kern3: np.ndarray, *, width=W, rpc=RPC, use_f32r=True,
                  psum_cols=2048, mm_cols=512, a_bufs=3,
                  out_bufs=2, psum_bufs=2, combine_engines=("vector",)):
    """Build the per-core Bass program. Shard layout: S[r] =
    padded_image[core_row0 + r], r in [0, rpc+2); out rows r in [0, rpc).

    The image tile a2 carries one zero column of padding on each side, so
    every tap's rhs slice [c0+i, c0+i+mm_cols) is in range and every matmul
    is full-width (satisfies the f32r even-count/8B-alignment ISA rules and
    needs no edge-chunk special cases)."""
    taps = _nonzero_taps(kern3)
    assert taps, "all-zero kernel handled host-side"

    jmin = min(j for j, _, _ in taps)
    jmax = max(j for j, _, _ in taps)
    nb = jmax - jmin  # extra rows of A needed beyond R
    R = (128 - nb) & ~1  # output rows per tile (even, for f32r ISA rules)

    # pulled-out tap: must be partition-aligned with A (j == jmin).
    pull_candidates = [t for t in taps if t[0] == jmin]
    pull_candidates.sort(key=lambda t: abs(t[1] - 1))
    pulled = pull_candidates[0]
    mm_taps = [t for t in taps if t != pulled]

    # group remaining taps by kernel column
    cols = {}
    for j, i, w in mm_taps:
        cols.setdefault(i, []).append((j, w))
    col_ids = sorted(cols.keys())

    mm_dt = F32R if use_f32r else F32

    nc = bacc.Bacc("TRN2", target_bir_lowering=False, debug=False,
                   num_devices=N_CORES)
    s_in = nc.dram_tensor("shard", [rpc + 2, width + 2], mm_dt, kind="ExternalInput").ap()
    out_d = nc.dram_tensor("out", [rpc, width], F32, kind="ExternalOutput").ap()
    bands_in = None
    if col_ids:
        bands_in = nc.dram_tensor(
            "bands", [len(col_ids), 128, 128], mm_dt, kind="ExternalInput"
        ).ap()

    # tiles of output rows
    tiles = []
    t = 0
    while t < rpc:
        r = min(R, rpc - t)
        tiles.append((t, r))
        t += r

    pj, pi, pw = pulled
    wp = width + 2  # padded tile width

    with tile.TileContext(nc) as tc:
        with (
            tc.tile_pool(name="bandp", bufs=1) as bandp,
            tc.tile_pool(name="ap", bufs=a_bufs) as apool,
            tc.tile_pool(name="op", bufs=out_bufs) as opool,
            tc.tile_pool(name="pp", bufs=psum_bufs, space="PSUM") as ppool,
        ):
            band_tiles = {}
            for ci, i in enumerate(col_ids):
                bt = bandp.tile([128, 128], mm_dt, tag=f"band{ci}")
                nc.sync.dma_start(out=bt, in_=bands_in[ci])
                band_tiles[i] = bt

            n_q = (width + psum_cols - 1) // psum_cols
            eng_i = 0
            for (t0, rt) in tiles:
                krows = rt + nb  # contraction rows for this tile
                a = apool.tile([128, wp], mm_dt, tag="a")
                nc.sync.dma_start(
                    out=a[0:krows, :],
                    in_=s_in[t0 + jmin: t0 + jmin + krows, :],
                )
                o = opool.tile([128, width], F32, tag="o")
                a_f = a.bitcast(F32) if use_f32r else a  # DVE view

                for q in range(n_q):
                    q0 = q * psum_cols
                    q1 = min(q0 + psum_cols, width)
                    if col_ids:
                        ps = ppool.tile([128, psum_cols], F32, tag="ps")
                        for c0 in range(q0, q1, mm_cols):
                            c1 = min(c0 + mm_cols, q1)
                            for ii, i in enumerate(col_ids):
                                # rhs cols [c0+i, c1+i) in padded coords
                                nc.tensor.matmul(
                                    out=ps[0:rt, c0 - q0:c1 - q0],
                                    lhsT=band_tiles[i][0:krows, 0:rt],
                                    rhs=a[0:krows, c0 + i:c1 + i],
                                    start=(ii == 0),
                                    stop=(ii == len(col_ids) - 1),
                                )
                    # drain psum -> out sbuf, fusing the pulled tap:
                    # out[:, x] = A[., x+pi-1]*pw + psum[:, x]
                    eng = getattr(nc, combine_engines[eng_i % len(combine_engines)])
                    eng_i += 1
                    if col_ids:
                        eng.scalar_tensor_tensor(
                            out=o[0:rt, q0:q1],
                            in0=a_f[0:rt, q0 + pi:q1 + pi],
                            scalar=pw,
                            in1=ps[0:rt, 0:q1 - q0],
                            op0=mybir.AluOpType.mult,
                            op1=mybir.AluOpType.add,
                        )
                    else:
                        eng.tensor_scalar_mul(
                            o[0:rt, q0:q1], a_f[0:rt, q0 + pi:q1 + pi], pw
                        )
                nc.sync.dma_start(out=out_d[t0: t0 + rt, :], in_=o[0:rt, :])

    nc.compile()

    meta = {
        "bands": (
            np.stack([
                _band_matrix(cols[i], jmin, 128, 128) for i in col_ids
            ]) if col_ids else None
        ),
    }
    return nc, meta


def kernel(image: np.ndarray, kernel: np.ndarray) -> np.ndarray:
    image = np.asarray(image)
    kernel = np.asarray(kernel, dtype=np.float32)
    img = np.ascontiguousarray(image.reshape(H, W).astype(np.float32))

    if not np.any(kernel):
        return np.zeros_like(image, dtype=np.float32).reshape(image.shape)

    nc, meta = build_program(kernel)

    padded = np.pad(img, ((1, 1), (1, 1)))
    in_maps = []
    for c in range(N_CORES):
        m = {"shard": np.ascontiguousarray(padded[c * RPC: c * RPC + RPC + 2])}
        if meta["bands"] is not None:
            m["bands"] = meta["bands"]
        in_maps.append(m)

    res = bass_utils.run_bass_kernel_spmd(nc, in_maps, core_ids=list(range(N_CORES)))
    out = np.concatenate([r["out"] for r in res.results], axis=0)
    return out.reshape(image.shape)

